# revision 12
# baseline (speedup 1.0000x reference)
"""Trainium2 Bass kernel for nn_PartialAttention (LN -> Q/K proj -> scaled QK^T -> exp(s - rowmax)).

Sharding: 8 cores = 2 batches x 4 query-blocks of 1024 queries.
Each core receives the full batch sequence in transposed layout xT = X_b^T
[E=1024, S=4096] (fp16), column-rolled so that its own query block occupies
columns 0..1023 (keeps the device program identical across cores).
The core computes LayerNorm statistics + K^T for the whole batch and
Q^T for its block via the decomposition

    K^T = r (.) (Wg_k^T xT) - sk (x) (r*mu) + ck (x) 1,   Wg_k = diag(gamma) Wk

then scores = Q^T.T @ K^T and out = exp(scores + EXP_BIAS), emitted as fp16.
The host divides each row by its max, which equals exp(s - rowmax) exactly
(tolerance 2e-2; this path measures ~1e-3). Device rowmax was removed after
v2 measured 9e-2 on HW (cross-bank PSUM reduce mis-read) and it costs 38us
of DVE anyway.

v4 structure, tuned for PE warmth (HAM re-throttles after ~3.4us idle):
- fp16 input/output/matmul streams; ~17 MiB DMA per core.
- S2 (sum of x^2) matmuls col-tiled at (0,96), concurrent with the M=65
  k-projection (strips 0-2) -> S2 stream time is hidden.
- rsqrt via int-magic seed + 2 Newton steps on DVE [128,16] tiles: no ACT
  Sqrt -> single exp table load, nothing on the do_half critical chain.
- phase interleave: after half 0 (chunks 0-3) the query block's scores
  for key banks 0-1 stream between chunks 4-7, filling the stats/epilogue
  PE gap; key banks 2-3 follow after half 1. Output rows DMA in halves.
"""

import os
from contextlib import ExitStack

import numpy as np

import concourse.bass as bass
import concourse.bacc as bacc
import concourse.mybir as mybir
import concourse.tile as tile
from concourse.bass import ts
from concourse.bass_utils import run_bass_kernel_spmd

F32 = mybir.dt.float32
F16 = mybir.dt.float16
I32 = mybir.dt.int32
FT = mybir.ActivationFunctionType
ALU = mybir.AluOpType

E, S, B, D = 1024, 4096, 2, 64
P = 128
NE = E // P            # 8 e-chunks of 128
TS = 512               # token chunk (= one fp32 PSUM bank)
NTS = S // TS          # 8
QB = 1024              # queries per core
NQC = QB // TS         # 2 ts-chunks belong to the query block
NQT = QB // P          # 8 query tiles of 128
EPS = 1e-5
SCALE = 1.0 / 8.0      # 1/sqrt(D)
EXP_BIAS = -2.0        # safety shift; cancels in host row-normalize
HK = S // 2            # keys covered per phase (2048)
MAGIC = 0x5F3759DF     # fp32 rsqrt seed

SQV = 7                # e-chunks squared on vector (rest on gpsimd)


def _body(tc, xT, wq, wk, gam, bet, bqv, bkv, cst, cneg, out):
    nc = tc.nc
    with ExitStack() as ctx:
        consts = ctx.enter_context(tc.tile_pool(name="consts", bufs=1))
        big = ctx.enter_context(tc.tile_pool(name="big", bufs=1))
        stats = ctx.enter_context(tc.tile_pool(name="stats", bufs=1))

        # ---------- parameter prep ----------
        wkt = consts.tile([P, NE, D], F16)
        nc.sync.dma_start(out=wkt, in_=wk.rearrange("(c p) d -> p c d", p=P))
        wqt = consts.tile([P, NE, D], F16)
        nc.sync.dma_start(out=wqt, in_=wq.rearrange("(c p) d -> p c d", p=P))
        gmt = consts.tile([P, NE], F32)
        nc.sync.dma_start(out=gmt, in_=gam)
        btt = consts.tile([P, NE], F16)
        nc.sync.dma_start(out=btt, in_=bet)
        bk_row = consts.tile([1, D], F32)
        nc.sync.dma_start(out=bk_row, in_=bkv.unsqueeze(0))
        bq_row = consts.tile([1, D], F32)
        nc.sync.dma_start(out=bq_row, in_=bqv.unsqueeze(0))

        wgk = consts.tile([P, NE, D + 1], F16)
        wgq = consts.tile([P, NE, D], F16)
        for c in range(NE):
            nc.vector.tensor_scalar_mul(wgk[:, c, 0:D], wkt[:, c, :], gmt[:, c : c + 1])
            nc.vector.tensor_scalar(
                wgq[:, c, :],
                wqt[:, c, :],
                gmt[:, c : c + 1],
                SCALE,
                op0=ALU.mult,
                op1=ALU.mult,
            )

        # cst[:, 0:15] = staircase (col NTS-1 ones), cst[:, 15] = ones.
        stair_ones = consts.tile([P, 2 * NTS], F16)
        nc.sync.dma_start(out=stair_ones, in_=cst)
        stair = stair_ones[:, 0 : 2 * NTS - 1]
        ones_col = stair_ones[:, 2 * NTS - 1 : 2 * NTS]
        ones_bcast = bass.AP(tensor=cst.tensor, offset=cst.offset + (2 * NTS - 1), ap=[[2 * NTS, P], [0, NE], [1, 1]])
        nc.sync.dma_start(out=wgk[:, :, D : D + 1], in_=ones_bcast)

        ebias = consts.tile([P, 1], F32)
        nc.vector.memset(ebias, EXP_BIAS)

        # sk/sq/ck/cq rows [1, D] via PE column sums; assembled into the
        # K=2 epilogue weights sk2 = [sk; ck], sq2 = [sq; cq].
        sk2 = consts.tile([2, D], F16)
        sq2 = consts.tile([2, D], F16)
        ck_row = consts.tile([1, D], F16)
        cq_row = consts.tile([1, D], F16)
        with tc.tile_pool(name="ppsum", bufs=1, space="PSUM") as pp:
            ps_par = pp.tile([1, 4 * D], F32)
            for g in range(4):
                for c in range(NE):
                    lhs = ones_col if g < 2 else btt[:, c : c + 1]
                    rhs_g = (wgk[:, c, 0:D], wgq[:, c, :], wkt[:, c, :], wqt[:, c, :])[g]
                    nc.tensor.matmul(ps_par[:, g * D : (g + 1) * D], lhsT=lhs, rhs=rhs_g, start=(c == 0), stop=(c == NE - 1), skip_group_check=True)
            nc.scalar.copy(sk2[0:1, :], ps_par[:, 0 * D : 1 * D])
            nc.scalar.copy(sq2[0:1, :], ps_par[:, 1 * D : 2 * D])
            nc.vector.tensor_add(ck_row, ps_par[:, 2 * D : 3 * D], bk_row)
            tmpc = stats.tile([1, D], F32)
            nc.vector.tensor_add(tmpc, ps_par[:, 3 * D : 4 * D], bq_row)
            nc.vector.tensor_scalar_mul(cq_row, tmpc, SCALE)
        nc.sync.dma_start(out=sk2[1:2, :], in_=ck_row)
        nc.sync.dma_start(out=sq2[1:2, :], in_=cq_row)

        # ---------- persistent SBUF ----------
        pkraw = big.tile([D + 1, S], F32)
        pqraw = big.tile([D, QB], F32)
        kT = big.tile([D, S], F16)
        qT = big.tile([D, QB], F16)
        rb = big.tile([D, S], F16)
        o_pairs = [big.tile([P, 2, S], F16, name=f"opair{i}") for i in range(NQT // 2)]
        rmu2 = consts.tile([2, S], F16)  # row0 = r*mu, row1 = -1
        nc.sync.dma_start(out=rmu2[1:2, :], in_=cneg)
        rdp = ctx.enter_context(tc.tile_pool(name="rdp", bufs=1, space="DRAM"))
        r_dram = rdp.tile([1, S], F16, name="r_scratch")
        rmu_dram = rdp.tile([1, S], F16, name="rmu_scratch")
        xT3 = xT.rearrange("(c p) t -> p c t", p=P)
        GROUPS = [(0, 4), (4, 4)]  # two equal chunk groups

        def out_half(m, lo, hi):
            """DMA rows [128m .. 128m+255] cols [lo:hi] from the pair tile."""
            opr = o_pairs[m // 2]
            dst = bass.AP(
                tensor=out.tensor,
                offset=out.offset + m * P * S + lo,
                ap=[[S, P], [P * S, 2], [1, hi - lo]],
            )
            nc.gpsimd.dma_start(out=dst, in_=opr[:, :, lo:hi])

        with (
            tc.tile_pool(name="xpool", bufs=2) as xpool,
            tc.tile_pool(name="sqpool", bufs=2) as sqpool,
            tc.tile_pool(name="kp", bufs=2, space="PSUM") as kp,
            tc.tile_pool(name="sp", bufs=1, space="PSUM") as sp,
            tc.tile_pool(name="ep", bufs=1, space="PSUM") as ep,
            tc.tile_pool(name="ktmp", bufs=2) as ktmp_pool,
            tc.tile_pool(name="scA", bufs=3, space="PSUM") as scA,
        ):
            ps_s2_halves = [
                sp.tile([P, TS], F32, name=f"ps_s2_{h}", tag=f"s2_{h}") for h in range(2)
            ]

            def chunk_work(j, xt2, u2):
                h = 0 if j < GROUPS[1][0] else 1
                jj = j - GROUPS[h][0]
                gn = GROUPS[h][1]
                xt = xt2[:, :, u2, :]
                xq2 = sqpool.tile([P, NE, TS], F16, name=f"xq2_{j}", tag="xq2")
                nc.vector.tensor_mul(xq2, xt, xt)

                pk = kp.tile([D + 1, TS], F32, name=f"pk{j}", tag="kq")
                for c in range(NE):
                    nc.tensor.matmul(pk, lhsT=wgk[:, c, :], rhs=xt[:, c, :], start=(c == 0), stop=(c == NE - 1))
                # S2 rides col strip 3 (cols 96..96+gn), concurrent with pk
                lhs_st = stair[:, NTS - 1 - jj : NTS - 1 - jj + gn]
                s2out = ps_s2_halves[h][96 : 96 + gn, :]
                for c in range(NE):
                    nc.tensor.matmul(s2out, lhsT=lhs_st, rhs=xq2[:, c, :], start=(jj == 0 and c == 0), stop=(jj == gn - 1 and c == NE - 1), skip_group_check=True, tile_position=(0, 96))
                if j < 4:
                    nc.scalar.copy(pkraw[:, ts(j, TS)], pk)
                else:
                    nc.vector.tensor_copy(pkraw[:, ts(j, TS)], pk)
                if j < NQC:
                    pq = kp.tile([D, TS], F32, name=f"pq{j}", tag="kq")
                    for c in range(NE):
                        nc.tensor.matmul(pq, lhsT=wgq[:, c, :], rhs=xt[:, c, :], start=(c == 0), stop=(c == NE - 1))
                    nc.scalar.copy(pqraw[:, ts(j, TS)], pq)

            def do_half(h):
                g0, gn = GROUPS[h]
                o = g0 * TS
                HT = gn * TS          # tokens in this half
                W = HT // P           # 16 per-partition stats columns
                # [128, W] layout (token = p*W + i): all DVE lanes active.
                s2s = stats.tile([P, TS], F32, name=f"s2s{h}", tag=f"s2s{h}")
                nc.scalar.copy(s2s[96 : 96 + gn, :], ps_s2_halves[h][96 : 96 + gn, :])
                s12 = stats.tile([P, 2 * W], F32, name=f"s12{h}", tag=f"s12{h}")
                nc.scalar.dma_start(out=s12[:, 0:W], in_=pkraw[D : D + 1, o : o + HT])
                nc.scalar.dma_start(out=s12[:, W : 2 * W], in_=s2s[96 : 96 + gn, :])
                muh = stats.tile([P, W], F32, name=f"muh{h}", tag=f"muh{h}")
                nc.vector.tensor_scalar_mul(muh, s12[:, 0:W], 1.0 / E)
                e2h = stats.tile([P, W], F32, name=f"e2h{h}", tag=f"e2h{h}")
                nc.vector.tensor_scalar_mul(e2h, s12[:, W : 2 * W], 1.0 / E)
                msqh = stats.tile([P, W], F32, name=f"msqh{h}", tag=f"msqh{h}")
                nc.vector.tensor_mul(msqh, muh, muh)
                veps = stats.tile([P, W], F32, name=f"veps{h}", tag=f"veps{h}")
                nc.vector.tensor_sub(veps, e2h, msqh)
                nc.vector.tensor_scalar_add(veps, veps, EPS)
                # rsqrt: int magic seed + 2 Newton iterations, all on DVE
                yh = stats.tile([P, W], F32, name=f"yh{h}", tag=f"yh{h}")
                yi = yh.bitcast(I32)
                nc.vector.tensor_scalar(yi, veps.bitcast(I32), 1, None, op0=ALU.logical_shift_right)
                nc.vector.tensor_scalar(yi, yi, -1, None, op0=ALU.bitwise_xor)
                nc.vector.tensor_scalar_add(yi, yi, MAGIC + 1)
                th = stats.tile([P, W], F32, name=f"th{h}", tag=f"th{h}")
                for _ in range(2):
                    nc.vector.tensor_mul(th, yh, yh)
                    nc.vector.tensor_mul(th, th, veps)
                    nc.vector.tensor_scalar(th, th, -0.5, 1.5, op0=ALU.mult, op1=ALU.add)
                    nc.vector.tensor_mul(yh, yh, th)
                rh = stats.tile([P, W], F16, name=f"rh{h}", tag=f"rh{h}")
                nc.vector.tensor_copy(rh, yh)
                rmuh = stats.tile([P, W], F16, name=f"rmuh{h}", tag=f"rmuh{h}")
                nc.vector.tensor_mul(rmuh, yh, muh)
                nc.scalar.dma_start(out=rmu_dram[0:1, o : o + HT], in_=rmuh)
                nc.scalar.dma_start(out=rmu2[0:1, o : o + HT], in_=rmu_dram[0:1, o : o + HT])
                nc.scalar.dma_start(out=r_dram[0:1, o : o + HT], in_=rh)
                r_bc = bass.AP(tensor=r_dram.tensor, offset=r_dram.offset + o, ap=[[0, D], [1, HT]])
                nc.scalar.dma_start(out=rb[:, o : o + HT], in_=r_bc)
                for j in range(g0, g0 + gn):
                    ob = ep.tile([D, TS], F32, name=f"ob{j}", tag="ob")
                    nc.tensor.matmul(ob, lhsT=sk2, rhs=rmu2[:, ts(j, TS)], start=True, stop=True)
                    tmp = ktmp_pool.tile([D, TS], F32, name=f"tmp{j}", tag="tmp")
                    nc.vector.tensor_mul(tmp, rb[:, ts(j, TS)], pkraw[0:D, ts(j, TS)])
                    nc.vector.tensor_sub(kT[:, ts(j, TS)], tmp, ob)
                    if j < NQC:
                        obq = ep.tile([D, TS], F32, name=f"obq{j}", tag="ob")
                        nc.tensor.matmul(obq, lhsT=sq2, rhs=rmu2[:, ts(j, TS)], start=True, stop=True)
                        tmpq = ktmp_pool.tile([D, TS], F32, name=f"tmpq{j}", tag="tmp")
                        nc.vector.tensor_mul(tmpq, rb[:, ts(j, TS)], pqraw[0:D, ts(j, TS)])
                        nc.vector.tensor_sub(qT[:, ts(j, TS)], tmpq, obq)

            def scores_a(m):
                o_sl = o_pairs[m // 2][:, m % 2, :]
                for u in range(HK // TS):
                    sc = scA.tile([P, TS], F32, name=f"sa{m}_{u}", tag="sa")
                    nc.tensor.matmul(sc, lhsT=qT[:, ts(m, P)], rhs=kT[:, ts(u, TS)], start=True, stop=True)
                    nc.scalar.activation(o_sl[:, ts(u, TS)], sc, FT.Exp, bias=ebias[:, 0:1])

            for jp in range(NTS // 2):
                xt2 = xpool.tile([P, NE, 2, TS], F16, name=f"xt{jp}", tag="xt")
                if jp == 0:
                    nc.sync.dma_start(out=xt2[:, :, 0, :], in_=xT3[:, :, 0:TS])
                    nc.sync.dma_start(out=xt2[:, :, 1, :], in_=xT3[:, :, TS : 2 * TS])
                else:
                    nc.sync.dma_start(out=xt2, in_=xT3[:, :, ts(jp, 2 * TS)])
                for u2 in range(2):
                    j = 2 * jp + u2
                    chunk_work(j, xt2, u2)
                    if j == 3:
                        do_half(0)
                    if 4 <= j <= 7:
                        m0 = 2 * (j - 4)
                        scores_a(m0)
                        scores_a(m0 + 1)
                        out_half(m0, 0, HK)
            do_half(1)

        # ---------- phase B: key banks 2-3 for every query tile ----------
        with tc.tile_pool(name="scB", bufs=2, space="PSUM") as scB:
            for m in range(NQT):
                sc = scB.tile([P, HK], F32, name=f"sb{m}", tag="sb")
                for u in range(HK // TS):
                    nc.tensor.matmul(sc[:, ts(u, TS)], lhsT=qT[:, ts(m, P)], rhs=kT[:, HK + u * TS : HK + (u + 1) * TS], start=True, stop=True)
                o_sl = o_pairs[m // 2][:, m % 2, :]
                nc.scalar.activation(o_sl[:, HK:S], sc, FT.Exp, bias=ebias[:, 0:1])
                if m % 2 == 1:
                    out_half(m - 1, HK, S)


def _build_nc():
    nc = bacc.Bacc("TRN2", target_bir_lowering=False, debug=False)
    xT = nc.dram_tensor("xT", [E, S], F16, kind="ExternalInput").ap()
    wq = nc.dram_tensor("Wq", [E, D], F16, kind="ExternalInput").ap()
    wk = nc.dram_tensor("Wk", [E, D], F16, kind="ExternalInput").ap()
    gam = nc.dram_tensor("gamma", [P, NE], F32, kind="ExternalInput").ap()
    bet = nc.dram_tensor("beta", [P, NE], F16, kind="ExternalInput").ap()
    bqv = nc.dram_tensor("bq", [D], F32, kind="ExternalInput").ap()
    bkv = nc.dram_tensor("bk", [D], F32, kind="ExternalInput").ap()
    cst = nc.dram_tensor("cst", [P, 2 * NTS], F16, kind="ExternalInput").ap()
    cneg = nc.dram_tensor("cneg", [1, S], F16, kind="ExternalInput").ap()
    out = nc.dram_tensor("out", [QB, S], F16, kind="ExternalOutput").ap()
    with tile.TileContext(nc) as tc:
        _body(tc, xT, wq, wk, gam, bet, bqv, bkv, cst, cneg, out)
    nc.compile()
    return nc


_nc_cache = None
_last_results = None


def kernel(src_emb, gamma, beta, Wq, bq, Wk, bk):
    global _nc_cache, _last_results
    src_emb = np.asarray(src_emb, np.float32)
    gamma = np.asarray(gamma, np.float32)
    beta = np.asarray(beta, np.float32)
    Wq = np.asarray(Wq, np.float32)
    bq = np.asarray(bq, np.float32)
    Wk = np.asarray(Wk, np.float32)
    bk = np.asarray(bk, np.float32)

    if _nc_cache is None:
        _nc_cache = _build_nc()
    nc = _nc_cache

    gamma_r = np.ascontiguousarray(gamma.reshape(NE, P).T)
    beta_r = np.ascontiguousarray(beta.reshape(NE, P).T).astype(np.float16)
    cst_np = np.zeros((P, 2 * NTS), np.float16)
    cst_np[:, NTS - 1] = 1.0
    cst_np[:, 2 * NTS - 1] = 1.0
    cneg_np = np.full((1, S), -1.0, np.float16)
    wq16 = Wq.astype(np.float16)
    wk16 = Wk.astype(np.float16)
    xT_all = np.ascontiguousarray(np.transpose(src_emb, (1, 2, 0)))  # [B, E, S]
    in_maps = []
    for c in range(8):
        b, qb = c // 4, c % 4
        s = qb * QB
        if s:
            xr = np.concatenate([xT_all[b][:, s:], xT_all[b][:, :s]], axis=1).astype(np.float16)
        else:
            xr = xT_all[b].astype(np.float16)
        in_maps.append({"xT": xr, "Wq": wq16, "Wk": wk16, "gamma": gamma_r, "beta": beta_r, "bq": bq, "bk": bk, "cst": cst_np, "cneg": cneg_np})

    res = run_bass_kernel_spmd(nc, in_maps, core_ids=list(range(8)))
    _last_results = res

    blocks = []
    for c in range(8):
        blk = np.asarray(res.results[c]["out"])
        s = (c % 4) * QB
        if s:
            blk = np.roll(blk, s, axis=1)
        blocks.append(blk)
    out16 = np.stack(
        [np.concatenate(blocks[0:4], axis=0), np.concatenate(blocks[4:8], axis=0)], axis=0
    )
    e = out16.astype(np.float32)
    # device emitted exp(s + EXP_BIAS); divide by the row max -> exp(s - rowmax)
    return e / e.max(axis=-1, keepdims=True)
